# revision 9
# baseline (speedup 1.0000x reference)
"""AdditiveAttention Trainium2 kernel (Bass/Tile), 8-core data-parallel.

Math (per batch b):
    q = queries @ Wq.T              [Q, H]
    k = keys @ Wk.T                 [K, H]
    scores[q,k] = sum_h Wv[h] * tanh(q[q,h] + k[k,h])
    attn = softmax(mask(scores))    positions >= valid_len -> -1e6
    out = attn @ values             [Q, V]

Device mapping (per core, 2 batches/core):
  - tanh stage on ScalarE: per (b, qi, h-tile) one ACT instr
    tanh(kT[h,ki] + bias=qT[h,qi]) over ki < VL32(b); the broadcast add is
    fused as the per-partition activation bias.  This is the roofline engine.
  - score reduction on PE: the fp16 tanh tile is the stationary operand
    (lhsT [h, ki-block], fast-weight-load eligible), rhs = Wv column [h,1];
    accumulates the two h-tiles into PSUM column (ki, qi), giving
    scores^T [ki, qi] per k-block with no layout fixup.
  - exp on ScalarE straight out of PSUM with bias = mask column
    (0 / -1e6 from valid_lens, computed on host); no max-subtraction
    (scores are O(1) by construction; masked lanes underflow to exactly 0).
  - AV on PE: lhsT = exp tile [ki, qi], rhs = [values | ones] [ki, 257];
    PSUM [qi, 257] accumulates over k-blocks, last column = softmax
    denominator.  Reciprocal + per-partition scale on VectorE.

Runtime specialization: kernel() reads valid_lens, rounds each batch's
K-extent up to a multiple of 32 (VL32), pairs batches so per-core work is
balanced, and builds one program per distinct (VL32_a, VL32_b) profile.
All computation for ki >= VL32 is skipped; the tail inside the padded
extent is masked exactly via the exp bias, and the never-written PSUM rows
of the last partial k-block are zeroed once per batch.
"""

import numpy as np

import jax
from jax.sharding import SingleDeviceSharding

import concourse.bass as bass
import concourse.mybir as mybir
import concourse.tile as tile
from concourse import bacc, bass2jax
from concourse.masks import make_identity

B, Q, K, H, V = 16, 128, 512, 256, 256
N_CORES = 8
B_LOC = B // N_CORES  # 2 batches per core
P = 128
HT = H // P   # 2 h-tiles
DT = H // P   # 2 d-tiles (projection contraction)
F32 = mybir.dt.float32
F16 = mybir.dt.float16
MASK_VAL = -1e6


def _emit(nc, tc, vls, queries_d, keys_d, values_d, wq_d, wk_d, wv_d, mask_d,
          out_d, ctx):
    const = ctx.enter_context(tc.tile_pool(name="const", bufs=1))
    stage = ctx.enter_context(tc.tile_pool(name="stage", bufs=2))
    persist = ctx.enter_context(tc.tile_pool(name="persist", bufs=1))
    twork = ctx.enter_context(tc.tile_pool(name="twork", bufs=6))
    ps_misc = ctx.enter_context(tc.tile_pool(name="ps_misc", bufs=2, space="PSUM"))
    ps_sc = ctx.enter_context(tc.tile_pool(name="ps_sc", bufs=2, space="PSUM"))
    ps_out = ctx.enter_context(tc.tile_pool(name="ps_out", bufs=2, space="PSUM"))

    ident = const.tile([P, P], F32)
    make_identity(nc, ident)

    # --- weights: load natural [h, d], PE-transpose into [d, h] ---
    wq_nat = const.tile([P, HT, H], F32)
    nc.sync.dma_start(out=wq_nat, in_=wq_d.rearrange("(t p) d -> p t d", p=P))
    wk_nat = const.tile([P, HT, H], F32)
    nc.sync.dma_start(out=wk_nat, in_=wk_d.rearrange("(t p) d -> p t d", p=P))
    wqT = const.tile([P, DT, H], F32)  # [d_in, dt, h]
    wkT = const.tile([P, DT, H], F32)
    for (w_nat, w_T) in ((wq_nat, wqT), (wk_nat, wkT)):
        for ht in range(HT):
            for dt in range(DT):
                ps = ps_misc.tile([P, P], F32, tag="ps_tr")
                nc.tensor.transpose(ps, w_nat[:, ht, dt * P:(dt + 1) * P], ident)
                nc.vector.tensor_copy(out=w_T[:, dt, ht * P:(ht + 1) * P], in_=ps)

    wv_sb = const.tile([P, HT], F16)
    nc.gpsimd.dma_start(out=wv_sb, in_=wv_d.rearrange("(t p) -> p t", p=P))
    mask_sb = const.tile([P, B_LOC, K // P], F32)
    nc.sync.dma_start(out=mask_sb, in_=mask_d.rearrange("b (kb p) -> p b kb", p=P))

    for b in range(B_LOC):
        vl32 = vls[b]              # padded K-extent, multiple of 32
        kb_n = (vl32 + P - 1) // P   # number of touched 128-blocks
        kpad = kb_n * P

        # --- load + transpose queries/keys, project ---
        q_nat = stage.tile([P, H], F32, tag="qnat")
        nc.sync.dma_start(out=q_nat, in_=queries_d[b])
        qTd = stage.tile([P, DT, Q], F32, tag="qTd")  # [d_in, dt, qi]
        for dt in range(DT):
            ps = ps_misc.tile([P, P], F32, tag="ps_tr")
            nc.tensor.transpose(ps, q_nat[:, dt * P:(dt + 1) * P], ident)
            nc.vector.tensor_copy(out=qTd[:, dt, :], in_=ps)

        k_nat = stage.tile([P, kb_n, H], F32, tag="knat")
        nc.sync.dma_start(
            out=k_nat,
            in_=keys_d[b, :kpad].rearrange("(kt p) d -> p kt d", p=P))
        kTd = stage.tile([P, DT, kpad], F32, tag="kTd")  # [d_in, dt, ki]
        for kt in range(kb_n):
            for dt in range(DT):
                ps = ps_misc.tile([P, P], F32, tag="ps_tr")
                nc.tensor.transpose(ps, k_nat[:, kt, dt * P:(dt + 1) * P], ident)
                nc.vector.tensor_copy(out=kTd[:, dt, kt * P:(kt + 1) * P], in_=ps)

        qT = persist.tile([P, HT, Q], F32, tag=f"qT{b}")  # [h_in, ht, qi]
        for ht in range(HT):
            ps = ps_misc.tile([P, P], F32, tag="ps_tr")
            for dt in range(DT):
                nc.tensor.matmul(ps, wqT[:, dt, ht * P:(ht + 1) * P], qTd[:, dt, :],
                                 start=(dt == 0), stop=(dt == DT - 1))
            nc.vector.tensor_copy(out=qT[:, ht, :], in_=ps)

        kT = persist.tile([P, HT, kpad], F32, tag=f"kT{b}")  # [h_in, ht, ki]
        for ht in range(HT):
            ps = ps_misc.tile([P, K], F32, tag="ps_prj")
            for dt in range(DT):
                nc.tensor.matmul(ps[:, :kpad], wkT[:, dt, ht * P:(ht + 1) * P],
                                 kTd[:, dt, :], start=(dt == 0), stop=(dt == DT - 1))
            nc.vector.tensor_copy(out=kT[:, ht, :], in_=ps[:, :kpad])

        # --- values with appended ones column ---
        vo = []
        for kb in range(kb_n):
            t = persist.tile([P, V + 1], F32, tag=f"vo{b}{kb}")
            nc.sync.dma_start(out=t[:, :V], in_=values_d[b, kb * P:(kb + 1) * P, :])
            nc.vector.memset(t[:, V:V + 1], 1.0)
            vo.append(t)

        # --- main loop: tanh + score columns ---
        # psum scores^T: [ki_in_block, kb*128 + qi]
        sc = ps_sc.tile([P, K], F32, tag="sc")
        rem = vl32 - (kb_n - 1) * P  # columns in last (possibly partial) block
        if rem < P:
            # rows never written by the score matmuls; exp must see 0 + mask.
            # (partition-offset writes are limited to 32-partition spans)
            for p0 in range(rem, P, 32):
                nc.vector.memset(sc[p0:p0 + 32, (kb_n - 1) * P:kb_n * P], 0.0)
        for qi in range(Q):
            tt = []
            for ht in range(HT):
                t = twork.tile([P, K], F16, tag="T")
                nc.scalar.activation(out=t[:, :vl32], in_=kT[:, ht, :vl32],
                                     func=mybir.ActivationFunctionType.Tanh,
                                     bias=qT[:, ht, qi:qi + 1])
                tt.append(t)
            for kb in range(kb_n):
                col = kb * P + qi
                cols = min(P, vl32 - kb * P)
                for ht in range(HT):
                    nc.tensor.matmul(sc[:cols, col:col + 1],
                                     tt[ht][:, kb * P:kb * P + cols],
                                     wv_sb[:, ht:ht + 1],
                                     start=(ht == 0), stop=(ht == HT - 1))

        # --- exp (+mask) and AV ---
        po = ps_out.tile([P, V + 1], F32, tag="po")
        for kb in range(kb_n):
            e = persist.tile([P, Q], F32, tag=f"E{b}{kb}")
            nc.scalar.activation(out=e, in_=sc[:, kb * P:(kb + 1) * P],
                                 func=mybir.ActivationFunctionType.Exp,
                                 bias=mask_sb[:, b, kb:kb + 1])
            nc.tensor.matmul(po, e, vo[kb], start=(kb == 0), stop=(kb == kb_n - 1))

        r = stage.tile([P, 1], F32, tag="recip")
        nc.vector.reciprocal(out=r, in_=po[:, V:V + 1])
        ot = stage.tile([P, V], F32, tag="ot")
        nc.vector.tensor_scalar_mul(ot, po[:, :V], r)
        nc.sync.dma_start(out=out_d[b], in_=ot)


def build_nc(vls, repeat=1):
    """vls: (vl32_slot0, vl32_slot1) padded K-extents for the two local batches."""
    from contextlib import ExitStack
    nc = bacc.Bacc("TRN2", target_bir_lowering=False, debug=False,
                   num_devices=N_CORES, enable_partition_id=False)
    queries_d = nc.dram_tensor("queries", [B_LOC, Q, H], F32, kind="ExternalInput").ap()
    keys_d = nc.dram_tensor("keys", [B_LOC, K, H], F32, kind="ExternalInput").ap()
    values_d = nc.dram_tensor("values", [B_LOC, K, V], F32, kind="ExternalInput").ap()
    wq_d = nc.dram_tensor("Wq", [H, H], F32, kind="ExternalInput").ap()
    wk_d = nc.dram_tensor("Wk", [H, H], F32, kind="ExternalInput").ap()
    wv_d = nc.dram_tensor("Wv", [H], F32, kind="ExternalInput").ap()
    mask_d = nc.dram_tensor("mask", [B_LOC, K], F32, kind="ExternalInput").ap()
    out_d = nc.dram_tensor("out", [B_LOC, Q, V], F32, kind="ExternalOutput").ap()

    with tile.TileContext(nc) as tc, ExitStack() as ctx:
        args = (nc, tc, vls, queries_d, keys_d, values_d, wq_d, wk_d, wv_d,
                mask_d, out_d, ctx)
        if repeat == 1:
            _emit(*args)
        else:
            with tc.For_i(0, repeat, 1):
                _emit(*args)
    nc.compile()
    return nc


def _make_single_core_runner(nc, device):
    """jit the program once for one device; reusable across calls."""
    bass2jax.install_neuronx_cc_hook()
    assert nc.partition_id_tensor is None
    in_names, out_names, out_avals, zero_shapes = [], [], [], []
    for alloc in nc.m.functions[0].allocations:
        if not isinstance(alloc, mybir.MemoryLocationSet):
            continue
        name = alloc.memorylocations[0].name
        if alloc.kind == "ExternalInput":
            in_names.append(name)
        elif alloc.kind == "ExternalOutput":
            shape = tuple(alloc.tensor_shape)
            npdt = np.dtype(mybir.dt.np(alloc.dtype))
            out_names.append(name)
            out_avals.append(jax.core.ShapedArray(shape, npdt))
            zero_shapes.append((shape, npdt))
    n_params = len(in_names)
    n_outs = len(out_avals)
    in_names_all = list(in_names) + list(out_names)

    def _body(*args):
        outs = bass2jax._bass_exec_p.bind(
            *args,
            out_avals=tuple(out_avals),
            in_names=tuple(in_names_all),
            out_names=tuple(out_names),
            lowering_input_output_aliases=(),
            sim_require_finite=True,
            sim_require_nnan=True,
            nc=nc,
        )
        return tuple(outs)

    fn = jax.jit(_body, donate_argnums=tuple(range(n_params, n_params + n_outs)),
                 keep_unused=True)
    sharding = SingleDeviceSharding(device)
    dev_in_cache = {}

    def launch(in_map):
        key = id(in_map)
        if key not in dev_in_cache:
            dev_in_cache.clear()
            dev_in_cache[key] = [
                jax.device_put(np.asarray(in_map[name]), sharding)
                for name in in_names
            ]
        args = list(dev_in_cache[key])
        args += [jax.device_put(np.zeros(s, d), sharding) for (s, d) in zero_shapes]
        outs = fn(*args)
        return dict(zip(out_names, outs))

    return launch


_NCS = {}       # (vls, repeat) -> compiled nc
_LAUNCH = {}    # (vls, repeat, core) -> launch fn


def _get_launch(vls, repeat, core):
    key = (vls, repeat, core)
    if key not in _LAUNCH:
        nckey = (vls, repeat)
        if nckey not in _NCS:
            _NCS[nckey] = build_nc(vls, repeat)
        _LAUNCH[key] = _make_single_core_runner(_NCS[nckey], jax.devices()[core])
    return _LAUNCH[key]


def plan_assignment(valid_lens):
    """Pair batches to balance per-core work; returns (perm, vls_per_core).

    perm[2c], perm[2c+1] are the global batch indices handled by core c.
    """
    vl32 = [min(K, (int(v) + 31) // 32 * 32) for v in valid_lens]
    order = sorted(range(B), key=lambda i: -vl32[i])
    perm, vls_per_core = [], []
    for c in range(N_CORES):
        a, b_ = order[c], order[2 * N_CORES - 1 - c]
        perm += [a, b_]
        vls_per_core.append((vl32[a], vl32[b_]))
    return perm, vls_per_core


def run_cores(in_maps, vls_per_core, repeat=1):
    """Launch all 8 per-core programs concurrently; returns per-core out dicts."""
    outs = [
        _get_launch(vls_per_core[c], repeat, c)(in_maps[c]) for c in range(N_CORES)
    ]
    jax.block_until_ready([list(o.values()) for o in outs])
    return [{k: np.asarray(v) for k, v in o.items()} for o in outs]


def make_in_maps(queries, keys, values, Wq, Wk, Wv, valid_lens, perm):
    queries = np.asarray(queries, np.float32)
    keys = np.asarray(keys, np.float32)
    values = np.asarray(values, np.float32)
    Wq = np.asarray(Wq, np.float32)
    Wk = np.asarray(Wk, np.float32)
    Wv = np.asarray(Wv, np.float32)
    valid_lens = np.asarray(valid_lens)
    mask = np.where(np.arange(K)[None, :] < valid_lens[:, None].astype(np.int64),
                    0.0, MASK_VAL).astype(np.float32)
    in_maps = []
    for c in range(N_CORES):
        ix = [perm[2 * c], perm[2 * c + 1]]
        in_maps.append({
            "queries": queries[ix], "keys": keys[ix], "values": values[ix],
            "Wq": Wq, "Wk": Wk, "Wv": Wv, "mask": mask[ix],
        })
    return in_maps


def kernel(queries, keys, values, Wq, Wk, Wv, valid_lens):
    perm, vls_per_core = plan_assignment(valid_lens)
    in_maps = make_in_maps(queries, keys, values, Wq, Wk, Wv, valid_lens, perm)
    res = run_cores(in_maps, vls_per_core)
    out = np.empty((B, Q, V), np.float32)
    for c in range(N_CORES):
        out[perm[2 * c]] = res[c]["out"][0]
        out[perm[2 * c + 1]] = res[c]["out"][1]
    return out
